# revision 10
# baseline (speedup 1.0000x reference)
"""Trainium2 Bass kernel for nn_DiagonalTraining (ragged per-anti-diagonal linear).

Math (reference): for each batch image x[b] (SxS) and each anti-diagonal
i (elements x[b, r, i-r], r=0..i), apply a per-diagonal linear layer:
  out[b,i,q] = sum_{r<=i} x[b,r,i-r] * W[i,q,r] + bias[i,q]   (q <= i)
and scatter back: y[b,q,i-q] = out[b,i,q]; positions with r+c >= S keep x.

Distribution: diagonal i -> core i%8, slot j=i//8 (64 slots per core,
balanced by construction). Host packs, per (core, slot), an augmented
matrix whose rows are the contraction axis r:
  [ D^T | V ]  with D^T[r,b]=x[b,r,i-r], V[r,q]=W[i,q,r]  (r,q < ni=i+1)
zero-padded to a core-independent size NJ=8*(j+1) so the SPMD program is
identical on all cores. The per-diagonal bias is added on the host while
scattering results back (elementwise, ~0.05% of the FLOPs).

The kernel is HBM-bound; the blob ships in bf16 (matmul accumulates in
f32 PSUM; rel-err ~2e-3, well under the 2e-2 gate) which halves traffic
vs f32. Each slot's contraction splits into <=128-row chunks. Remainder
chunks (<128 rows) are stacked vertically inside one 128-partition
column block at PE-legal partition offsets (pb in {0,32,64,96} honoring
the tile_position alignment rules), reclaiming most of the row-padding
zeros. All windows live in SBUF simultaneously (no recycling, ~100KB of
the 208KB per partition), so every window DMA is issued up front with
no WAR dependencies, round-robined over the three independent DMA
queues (sync/HWDGE, scalar/HWDGE, gpsimd/SWDGE) so per-DMA completion
latency overlaps other queues' data movement. Matmuls accumulate
psum[32, NJ] per slot inside a bank-packed 4-slot group psum tile; one
DVE copy per group casts to a bf16 staging tile and the group's slice
streams out immediately on a rotating queue.
"""

import sys

for _p in ("/opt/trn_rl_repo", "/opt/pypackages"):
    if _p not in sys.path:
        sys.path.append(_p)

import numpy as np

import concourse.bass as bass  # noqa: F401
import concourse.tile as tile
from concourse import bacc, mybir
from concourse.bass_utils import run_bass_kernel_spmd

B = 32          # batch
S = 512         # seq len / number of diagonals
N_CORES = 8
N_SLOTS = S // N_CORES  # 64 slots per core
DCOL = B        # width of the D^T block (batch on matmul M axis)
GROUP = 4       # slots per psum group
N_GROUPS = N_SLOTS // GROUP

KCFG = {
    "compute": "bf16",   # "f32" | "f32r" | "bf16"
    "out": "bf16",       # outblob dtype: "f32" | "bf16"
    "psum_bufs": 2,
    # big-window capacities in elements-per-partition (first windows
    # small so the first matmuls start early; then big for DMA rate)
    "wcaps": (2048, 4096),
    "wcap_rest": 8192,
}

# ---- static layout ----------------------------------------------------
# processing order: largest slot first
_ORDER = list(range(N_SLOTS - 1, -1, -1))
_GROUPS = [_ORDER[g * GROUP : (g + 1) * GROUP] for g in range(N_GROUPS)]


def _wcap(w):
    caps = KCFG["wcaps"]
    return caps[w] if w < len(caps) else KCFG["wcap_rest"]


# Windows are (height, width) SBUF tiles shipped by one DMA each.
# Full 128-row chunks pack side by side into [128, wcap] big windows.
# Remainder chunks (rows = 8m < 128, m = n % 16) go into one shared
# [8m, W_m] "bucket" window per height m — the DMA ships only the live
# rows, so remainder row-padding zeros are never shipped, while matmuls
# still read at partition base 0 (nonzero bases crash walrus/HW).
_SLOT_CHUNKS = {}   # j -> list of (win, cbase, pb, rows, row_start)
_WIN_H = []         # window heights
_WIN_W = []         # window widths (exact used, no tail padding)
_cur_win = None
_n_big = 0          # ordinal of the current big window (for _wcap)
_bucket_win = {}    # m -> window id

for _j in _ORDER:
    _n = _j + 1
    _NJ = 8 * _n
    _wd = DCOL + _NJ
    _chs = []
    _nfull = _NJ // 128
    for _c in range(_nfull):
        if _cur_win is None or _WIN_W[_cur_win] + _wd > _wcap(_n_big - 1):
            _WIN_H.append(128)
            _WIN_W.append(0)
            _cur_win = len(_WIN_H) - 1
            _n_big += 1
        _chs.append((_cur_win, _WIN_W[_cur_win], 0, 128, 128 * _c))
        _WIN_W[_cur_win] += _wd
    _rem = _NJ - 128 * _nfull
    if _rem:
        _m = _n % 16
        if _m not in _bucket_win:
            _WIN_H.append(_rem)
            _WIN_W.append(0)
            _bucket_win[_m] = len(_WIN_H) - 1
        _bw = _bucket_win[_m]
        _chs.append((_bw, _WIN_W[_bw], 0, _rem, 128 * _nfull))
        _WIN_W[_bw] += _wd
    _SLOT_CHUNKS[_j] = _chs
N_WINS = len(_WIN_H)
_WIN_OFF = []
_boff = 0
for _w in range(N_WINS):
    _WIN_OFF.append(_boff)
    _boff += _WIN_H[_w] * _WIN_W[_w]
BLOB_ELEMS = _boff

# DMA issue order: first big window first (first matmuls need it), then
# every bucket (they serve slots early in the processing order and total
# only ~2.4MB), then the remaining big windows.
_BIG_WINS = [w for w in range(N_WINS) if _WIN_H[w] == 128]
_BUCKET_WINS = sorted(
    (w for w in range(N_WINS) if _WIN_H[w] < 128),
    key=lambda w: -_WIN_H[w],
)
_ISSUE_ORDER = _BIG_WINS[:1] + _BUCKET_WINS + _BIG_WINS[1:]

# psum group column layout (bank-aligned so no matmul straddles a bank;
# the alignment gaps are never read — copies are per-slot)
_BANK = 512
_GROUP_COLS = []
_GROUP_W = []
for _slots in _GROUPS:
    _col = 0
    _cols = []
    for _j in _slots:
        _NJ = 8 * (_j + 1)
        if _col // _BANK != (_col + _NJ - 1) // _BANK:
            _col = ((_col + _BANK - 1) // _BANK) * _BANK
        _cols.append((_j, _col))
        _col += _NJ
    _GROUP_COLS.append(_cols)
    _GROUP_W.append(_col)

# acc/outblob layout: gap-free prefix-sum of NJ in processing order
_ACC_COL = {}       # j -> acc column
_GROUP_ACOL = []    # g -> (start col, width)
_acol = 0
for _g in range(N_GROUPS):
    _g0 = _acol
    for _j in _GROUPS[_g]:
        _ACC_COL[_j] = _acol
        _acol += 8 * (_j + 1)
    _GROUP_ACOL.append((_g0, _acol - _g0))
TOT_W = _acol
OUT_ELEMS = B * TOT_W

_compiled_nc = None


def _build_program():
    global _compiled_nc
    if _compiled_nc is not None:
        return _compiled_nc

    from contextlib import ExitStack

    nc = bacc.Bacc("TRN2", target_bir_lowering=False, debug=False)
    f32 = mybir.dt.float32
    mm_dt = {
        "f32": f32,
        "f32r": mybir.dt.float32r,
        "bf16": mybir.dt.bfloat16,
    }[KCFG["compute"]]
    out_dt = {"f32": f32, "bf16": mybir.dt.bfloat16}[KCFG["out"]]
    blob = nc.dram_tensor("blob", [BLOB_ELEMS], mm_dt, kind="ExternalInput").ap()
    outb = nc.dram_tensor("outblob", [OUT_ELEMS], out_dt, kind="ExternalOutput").ap()

    with tile.TileContext(nc) as tc, ExitStack() as ctx:
        win_pool = ctx.enter_context(tc.tile_pool(name="win", bufs=1))
        acc_pool = ctx.enter_context(tc.tile_pool(name="acc", bufs=1))
        psum_pool = ctx.enter_context(
            tc.tile_pool(name="psum", bufs=KCFG["psum_bufs"], space="PSUM")
        )

        qs = [nc.sync, nc.scalar, nc.gpsimd]

        # all windows live simultaneously; all loads issued up front with
        # no WAR deps, round-robined over the three DMA queues
        win_tiles = [None] * N_WINS
        for qi, w in enumerate(_ISSUE_ORDER):
            h, wf = _WIN_H[w], _WIN_W[w]
            t = win_pool.tile([h, wf], mm_dt, name=f"win{w}", tag=f"w{w}")
            src = blob[_WIN_OFF[w] : _WIN_OFF[w] + h * wf].rearrange(
                "(p f) -> p f", p=h, f=wf
            )
            qs[qi % 3].dma_start(t[:], src)
            win_tiles[w] = t

        acc_t = acc_pool.tile([B, TOT_W], out_dt)
        dstv = outb[:].rearrange("(p w) -> p w", p=B, w=TOT_W)
        nq = N_WINS
        for g, slots in enumerate(_GROUPS):
            gw = _GROUP_W[g]
            psum_t = psum_pool.tile([B, gw], f32)
            for j, col in _GROUP_COLS[g]:
                NJ = 8 * (j + 1)
                wd = DCOL + NJ
                chs = _SLOT_CHUNKS[j]
                for c, (w, cb, pb, rows, _rs) in enumerate(chs):
                    t = win_tiles[w]
                    nc.tensor.matmul(
                        psum_t[:, col : col + NJ],
                        t[pb : pb + rows, cb : cb + DCOL],
                        t[pb : pb + rows, cb + DCOL : cb + wd],
                        start=(c == 0),
                        stop=(c == len(chs) - 1),
                    )
            # per-slot copies: skip psum bank-alignment gaps (never
            # written), land gap-free in acc
            for j, col in _GROUP_COLS[g]:
                NJ = 8 * (j + 1)
                ac = _ACC_COL[j]
                nc.vector.tensor_copy(
                    acc_t[:, ac : ac + NJ], psum_t[:, col : col + NJ]
                )
            # stream this group's slice out; store waits only block later
            # (even more dependent) stores on the same queue
            ga, gaw = _GROUP_ACOL[g]
            qs[nq % 3].dma_start(dstv[:, ga : ga + gaw], acc_t[:, ga : ga + gaw])
            nq += 1

    nc.compile()
    _compiled_nc = nc
    return nc


def _np_dt():
    if KCFG["compute"] == "bf16":
        import ml_dtypes

        return ml_dtypes.bfloat16
    return np.float32


def _pack_core(k, x, W, bias):
    np_dt = _np_dt()
    blob = np.zeros(BLOB_ELEMS, np_dt)
    for j in range(N_SLOTS):
        i = N_CORES * j + k
        ni = i + 1
        NJ = 8 * (j + 1)
        wd = DCOL + NJ
        M = np.zeros((NJ, wd), np.float32)
        r = np.arange(ni)
        M[:ni, :DCOL] = x[:, r, i - r].T               # D^T[r, b]
        M[:ni, DCOL : DCOL + ni] = W[i, :ni, :ni].T    # V[r, q]
        Mc = M.astype(np_dt)
        for w, cb, pb, rows, rs in _SLOT_CHUNKS[j]:
            rl = Mc[rs : rs + rows]                    # [rows, wd]
            h, wf = _WIN_H[w], _WIN_W[w]
            img = blob[_WIN_OFF[w] : _WIN_OFF[w] + h * wf].reshape(h, wf)
            img[pb : pb + rows, cb : cb + wd] = rl
    return blob


def kernel(x, W, b):
    x = np.asarray(x, np.float32)
    W = np.asarray(W, np.float32)
    b = np.asarray(b, np.float32)

    nc = _build_program()
    in_maps = [{"blob": _pack_core(k, x, W, b)} for k in range(N_CORES)]
    res = run_bass_kernel_spmd(nc, in_maps, list(range(N_CORES)))

    y = x.copy()
    for k in range(N_CORES):
        ob = np.asarray(res.results[k]["outblob"], np.float32).reshape(B, TOT_W)
        for j in range(N_SLOTS):
            i = N_CORES * j + k
            ni = i + 1
            ac = _ACC_COL[j]
            q = np.arange(ni)
            y[:, q, i - q] = ob[:, ac : ac + ni] + b[i, :ni][None]
    return y


# revision 13
# speedup vs baseline: 1.2286x; 1.2286x over previous
"""Trainium2 Bass kernel for nn_DiagonalTraining (ragged per-anti-diagonal linear).

Math (reference): for each batch image x[b] (SxS) and each anti-diagonal
i (elements x[b, r, i-r], r=0..i), apply a per-diagonal linear layer:
  out[b,i,q] = sum_{r<=i} x[b,r,i-r] * W[i,q,r] + bias[i,q]   (q <= i)
and scatter back: y[b,q,i-q] = out[b,i,q]; positions with r+c >= S keep x.

Distribution: diagonal i -> core i%8, slot j=i//8 (64 slots per core,
balanced by construction). Host packs, per (core, slot), an augmented
matrix whose rows are the contraction axis r:
  [ D^T | V ]  with D^T[r,b]=x[b,r,i-r], V[r,q]=W[i,q,r]  (r,q < ni=i+1)
zero-padded to a core-independent size NJ=8*(j+1) so the SPMD program is
identical on all cores. The per-diagonal bias is added on the host while
scattering results back (elementwise, ~0.05% of the FLOPs).

The kernel is HBM-bound; the blob ships in bf16 (matmul accumulates in
f32 PSUM; rel-err ~2e-3, well under the 2e-2 gate) which halves traffic
vs f32. Each slot's contraction splits into <=128-row chunks. Remainder
chunks (<128 rows) are stacked vertically inside one 128-partition
column block at PE-legal partition offsets (pb in {0,32,64,96} honoring
the tile_position alignment rules), reclaiming most of the row-padding
zeros. All windows live in SBUF simultaneously (no recycling, ~100KB of
the 208KB per partition), so every window DMA is issued up front with
no WAR dependencies, round-robined over the three independent DMA
queues (sync/HWDGE, scalar/HWDGE, gpsimd/SWDGE) so per-DMA completion
latency overlaps other queues' data movement. Matmuls accumulate
psum[32, NJ] per slot inside a bank-packed 4-slot group psum tile; one
DVE copy per group casts to a bf16 staging tile and the group's slice
streams out immediately on a rotating queue.
"""

import sys

for _p in ("/opt/trn_rl_repo", "/opt/pypackages"):
    if _p not in sys.path:
        sys.path.append(_p)

import numpy as np

import concourse.bass as bass  # noqa: F401
import concourse.tile as tile
from concourse import bacc, mybir
from concourse.bass_utils import run_bass_kernel_spmd

B = 32          # batch
S = 512         # seq len / number of diagonals
N_CORES = 8
N_SLOTS = S // N_CORES  # 64 slots per core
DCOL = B        # width of the D^T block (batch on matmul M axis)
GROUP = 4       # slots per psum group
N_GROUPS = N_SLOTS // GROUP

KCFG = {
    "compute": "bf16",   # "f32" | "f32r" | "bf16"
    "out": "bf16",       # outblob dtype: "f32" | "bf16"
    "psum_bufs": 2,
    # window capacities in elements-per-partition (first windows small
    # so the first matmuls start early; then big for DMA rate). Total
    # window count must stay <= 8: the tile scheduler has 8 DMA
    # completion semaphore lanes, and lane reuse couples a DMA to the
    # consumers of the DMA eight issues earlier (measured 38us stall).
    "wcaps": (2048, 4096),
    "wcap_rest": 8960,
    # store stage boundaries (after these psum groups) — 4 staged
    # stores keep total DMA count at 12
    "store_stages": (5, 9, 12, 15),
}

# ---- static layout ----------------------------------------------------
# processing order: largest slot first
_ORDER = list(range(N_SLOTS - 1, -1, -1))
_GROUPS = [_ORDER[g * GROUP : (g + 1) * GROUP] for g in range(N_GROUPS)]


def _wcap(w):
    caps = KCFG["wcaps"]
    return caps[w] if w < len(caps) else KCFG["wcap_rest"]


# Windows are [128, width] SBUF tiles shipped by one DMA each; every
# chunk (including sub-128-row remainders, zero-padded to 128 rows)
# gets its own column block. Matmul operands must sit at partition
# base 0 (nonzero bases crash walrus/HW codegen).
_SLOT_CHUNKS = {}   # j -> list of (win, cbase, pb, rows, row_start)
_WIN_W = []         # window widths (exact used, no tail padding)
_cur_win = None

for _j in _ORDER:
    _n = _j + 1
    _NJ = 8 * _n
    _wd = DCOL + _NJ
    _chs = []
    for _c in range(-(-_NJ // 128)):
        _rows = min(128, _NJ - 128 * _c)
        if _cur_win is None or _WIN_W[_cur_win] + _wd > _wcap(_cur_win):
            _WIN_W.append(0)
            _cur_win = len(_WIN_W) - 1
        _chs.append((_cur_win, _WIN_W[_cur_win], 0, _rows, 128 * _c))
        _WIN_W[_cur_win] += _wd
    _SLOT_CHUNKS[_j] = _chs
N_WINS = len(_WIN_W)
assert N_WINS <= 8, f"{N_WINS} windows would reuse DMA semaphore lanes"
_WIN_H = [128] * N_WINS
_WIN_OFF = []
_boff = 0
for _w in range(N_WINS):
    _WIN_OFF.append(_boff)
    _boff += 128 * _WIN_W[_w]
BLOB_ELEMS = _boff

# psum group column layout (bank-aligned so no matmul straddles a bank;
# the alignment gaps are never read — copies are per-slot)
_BANK = 512
_GROUP_COLS = []
_GROUP_W = []
for _slots in _GROUPS:
    _col = 0
    _cols = []
    for _j in _slots:
        _NJ = 8 * (_j + 1)
        if _col // _BANK != (_col + _NJ - 1) // _BANK:
            _col = ((_col + _BANK - 1) // _BANK) * _BANK
        _cols.append((_j, _col))
        _col += _NJ
    _GROUP_COLS.append(_cols)
    _GROUP_W.append(_col)

# acc/outblob layout: gap-free prefix-sum of NJ in processing order
_ACC_COL = {}       # j -> acc column
_GROUP_ACOL = []    # g -> (start col, width)
_acol = 0
for _g in range(N_GROUPS):
    _g0 = _acol
    for _j in _GROUPS[_g]:
        _ACC_COL[_j] = _acol
        _acol += 8 * (_j + 1)
    _GROUP_ACOL.append((_g0, _acol - _g0))
TOT_W = _acol
OUT_ELEMS = B * TOT_W

_compiled_nc = None


def _build_program():
    global _compiled_nc
    if _compiled_nc is not None:
        return _compiled_nc

    from contextlib import ExitStack

    nc = bacc.Bacc("TRN2", target_bir_lowering=False, debug=False)
    f32 = mybir.dt.float32
    mm_dt = {
        "f32": f32,
        "f32r": mybir.dt.float32r,
        "bf16": mybir.dt.bfloat16,
    }[KCFG["compute"]]
    out_dt = {"f32": f32, "bf16": mybir.dt.bfloat16}[KCFG["out"]]
    blob = nc.dram_tensor("blob", [BLOB_ELEMS], mm_dt, kind="ExternalInput").ap()
    outb = nc.dram_tensor("outblob", [OUT_ELEMS], out_dt, kind="ExternalOutput").ap()

    with tile.TileContext(nc) as tc, ExitStack() as ctx:
        win_pool = ctx.enter_context(tc.tile_pool(name="win", bufs=1))
        acc_pool = ctx.enter_context(tc.tile_pool(name="acc", bufs=1))
        psum_pool = ctx.enter_context(
            tc.tile_pool(name="psum", bufs=KCFG["psum_bufs"], space="PSUM")
        )

        # loads and stores ride sync + gpsimd; the scalar (Act) engine is
        # reserved for half the psum->acc copies
        dqs = [nc.sync, nc.gpsimd]

        # all windows live simultaneously; all loads issued up front with
        # no WAR deps, alternating over the two DMA queues
        win_tiles = []
        for w in range(N_WINS):
            wf = _WIN_W[w]
            t = win_pool.tile([128, wf], mm_dt, name=f"win{w}", tag=f"w{w}")
            src = blob[_WIN_OFF[w] : _WIN_OFF[w] + 128 * wf].rearrange(
                "(p f) -> p f", p=128, f=wf
            )
            dqs[w % 2].dma_start(t[:], src)
            win_tiles.append(t)

        acc_t = acc_pool.tile([B, TOT_W], out_dt)
        dstv = outb[:].rearrange("(p w) -> p w", p=B, w=TOT_W)
        ncopy = 0
        nst = 0
        st_from = 0
        for g, slots in enumerate(_GROUPS):
            gw = _GROUP_W[g]
            psum_t = psum_pool.tile([B, gw], f32)
            for j, col in _GROUP_COLS[g]:
                NJ = 8 * (j + 1)
                wd = DCOL + NJ
                chs = _SLOT_CHUNKS[j]
                for c, (w, cb, pb, rows, _rs) in enumerate(chs):
                    t = win_tiles[w]
                    nc.tensor.matmul(
                        psum_t[:, col : col + NJ],
                        t[pb : pb + rows, cb : cb + DCOL],
                        t[pb : pb + rows, cb + DCOL : cb + wd],
                        start=(c == 0),
                        stop=(c == len(chs) - 1),
                    )
            # per-slot copies (skip psum bank-alignment gaps, land
            # gap-free in acc), alternating vector / scalar engines
            for j, col in _GROUP_COLS[g]:
                NJ = 8 * (j + 1)
                ac = _ACC_COL[j]
                eng = nc.vector if ncopy % 2 == 0 else nc.scalar
                if ncopy % 2 == 0:
                    eng.tensor_copy(
                        acc_t[:, ac : ac + NJ], psum_t[:, col : col + NJ]
                    )
                else:
                    eng.copy(acc_t[:, ac : ac + NJ], psum_t[:, col : col + NJ])
                ncopy += 1
            # staged stores: flush finished acc ranges while later
            # groups still compute
            if g in KCFG["store_stages"]:
                ga, gaw = _GROUP_ACOL[g]
                st_to = ga + gaw
                dqs[nst % 2].dma_start(
                    dstv[:, st_from:st_to], acc_t[:, st_from:st_to]
                )
                st_from = st_to
                nst += 1

    nc.compile()
    _compiled_nc = nc
    return nc


def _np_dt():
    if KCFG["compute"] == "bf16":
        import ml_dtypes

        return ml_dtypes.bfloat16
    return np.float32


def _pack_core(k, x, W, bias):
    np_dt = _np_dt()
    blob = np.zeros(BLOB_ELEMS, np_dt)
    for j in range(N_SLOTS):
        i = N_CORES * j + k
        ni = i + 1
        NJ = 8 * (j + 1)
        wd = DCOL + NJ
        M = np.zeros((NJ, wd), np.float32)
        r = np.arange(ni)
        M[:ni, :DCOL] = x[:, r, i - r].T               # D^T[r, b]
        M[:ni, DCOL : DCOL + ni] = W[i, :ni, :ni].T    # V[r, q]
        Mc = M.astype(np_dt)
        for w, cb, pb, rows, rs in _SLOT_CHUNKS[j]:
            rl = Mc[rs : rs + rows]                    # [rows, wd]
            h, wf = _WIN_H[w], _WIN_W[w]
            img = blob[_WIN_OFF[w] : _WIN_OFF[w] + h * wf].reshape(h, wf)
            img[pb : pb + rows, cb : cb + wd] = rl
    return blob


def kernel(x, W, b):
    x = np.asarray(x, np.float32)
    W = np.asarray(W, np.float32)
    b = np.asarray(b, np.float32)

    nc = _build_program()
    in_maps = [{"blob": _pack_core(k, x, W, b)} for k in range(N_CORES)]
    res = run_bass_kernel_spmd(nc, in_maps, list(range(N_CORES)))

    y = x.copy()
    for k in range(N_CORES):
        ob = np.asarray(res.results[k]["outblob"], np.float32).reshape(B, TOT_W)
        for j in range(N_SLOTS):
            i = N_CORES * j + k
            ni = i + 1
            ac = _ACC_COL[j]
            q = np.arange(ni)
            y[:, q, i - q] = ob[:, ac : ac + ni] + b[i, :ni][None]
    return y


# revision 16
# speedup vs baseline: 1.3810x; 1.1241x over previous
"""Trainium2 Bass kernel for nn_DiagonalTraining (ragged per-anti-diagonal linear).

Math (reference): for each batch image x[b] (SxS) and each anti-diagonal
i (elements x[b, r, i-r], r=0..i), apply a per-diagonal linear layer:
  out[b,i,q] = sum_{r<=i} x[b,r,i-r] * W[i,q,r] + bias[i,q]   (q <= i)
and scatter back: y[b,q,i-q] = out[b,i,q]; positions with r+c >= S keep x.

Distribution: diagonal i -> core i%8, slot j=i//8 (64 slots per core,
balanced by construction). Host packs, per (core, slot), an augmented
matrix whose rows are the contraction axis r:
  [ D^T | V ]  with D^T[r,b]=x[b,r,i-r], V[r,q]=W[i,q,r]  (r,q < ni=i+1)
zero-padded to a core-independent size NJ=8*(j+1) so the SPMD program is
identical on all cores. The per-diagonal bias is added on the host while
scattering results back (elementwise, ~0.05% of the FLOPs).

The kernel is HBM-bound; the blob ships in bf16 (matmul accumulates in
f32 PSUM; rel-err ~2e-3, well under the 2e-2 gate) which halves traffic
vs f32. Each slot's contraction splits into <=128-row chunks. Remainder
chunks (<128 rows) are stacked vertically inside one 128-partition
column block at PE-legal partition offsets (pb in {0,32,64,96} honoring
the tile_position alignment rules), reclaiming most of the row-padding
zeros. All windows live in SBUF simultaneously (no recycling, ~100KB of
the 208KB per partition), so every window DMA is issued up front with
no WAR dependencies, round-robined over the three independent DMA
queues (sync/HWDGE, scalar/HWDGE, gpsimd/SWDGE) so per-DMA completion
latency overlaps other queues' data movement. Matmuls accumulate
psum[32, NJ] per slot inside a bank-packed 4-slot group psum tile; one
DVE copy per group casts to a bf16 staging tile and the group's slice
streams out immediately on a rotating queue.
"""

import sys

for _p in ("/opt/trn_rl_repo", "/opt/pypackages"):
    if _p not in sys.path:
        sys.path.append(_p)

import numpy as np

import concourse.bass as bass  # noqa: F401
import concourse.tile as tile
from concourse import bacc, mybir
from concourse.bass_utils import run_bass_kernel_spmd

B = 32          # batch
S = 512         # seq len / number of diagonals
N_CORES = 8
N_SLOTS = S // N_CORES  # 64 slots per core
DCOL = B        # width of the D^T block (batch on matmul M axis)
GROUP = 4       # slots per psum group
N_GROUPS = N_SLOTS // GROUP

KCFG = {
    "compute": "bf16",   # "f32" | "f32r" | "bf16"
    "out": "bf16",       # outblob dtype: "f32" | "bf16"
    "psum_bufs": 2,
    # window capacities in elements-per-partition (first windows small
    # so the first matmuls start early; then big for DMA rate). Total
    # window count must stay <= 8: the tile scheduler has 8 DMA
    # completion semaphore lanes, and lane reuse couples a DMA to the
    # consumers of the DMA eight issues earlier (measured 38us stall).
    "wcaps": (2048, 3072, 6144),
    "wcap_rest": 9984,
}

# ---- static layout ----------------------------------------------------
# processing order: largest slot first
_ORDER = list(range(N_SLOTS - 1, -1, -1))
_GROUPS = [_ORDER[g * GROUP : (g + 1) * GROUP] for g in range(N_GROUPS)]


def _wcap(w):
    caps = KCFG["wcaps"]
    return caps[w] if w < len(caps) else KCFG["wcap_rest"]


# Windows are [128, width] SBUF tiles shipped by one DMA each; every
# chunk (including sub-128-row remainders, zero-padded to 128 rows)
# gets its own column block. Matmul operands must sit at partition
# base 0 (nonzero bases crash walrus/HW codegen).
_SLOT_CHUNKS = {}   # j -> list of (win, cbase, pb, rows, row_start)
_WIN_W = []         # window widths (exact used, no tail padding)
_cur_win = None

for _j in _ORDER:
    _n = _j + 1
    _NJ = 8 * _n
    _wd = DCOL + _NJ
    _chs = []
    for _c in range(-(-_NJ // 128)):
        _rows = min(128, _NJ - 128 * _c)
        if _cur_win is None or _WIN_W[_cur_win] + _wd > _wcap(_cur_win):
            _WIN_W.append(0)
            _cur_win = len(_WIN_W) - 1
        _chs.append((_cur_win, _WIN_W[_cur_win], 0, _rows, 128 * _c))
        _WIN_W[_cur_win] += _wd
    _SLOT_CHUNKS[_j] = _chs
N_WINS = len(_WIN_W)
assert N_WINS <= 8, f"{N_WINS} windows would reuse DMA semaphore lanes"
_WIN_H = [128] * N_WINS
_WIN_OFF = []
_boff = 0
for _w in range(N_WINS):
    _WIN_OFF.append(_boff)
    _boff += 128 * _WIN_W[_w]
BLOB_ELEMS = _boff

# psum group column layout (bank-aligned so no matmul straddles a bank;
# the alignment gaps are never read — copies are per-slot)
_BANK = 512
_GROUP_COLS = []
_GROUP_W = []
for _slots in _GROUPS:
    _col = 0
    _cols = []
    for _j in _slots:
        _NJ = 8 * (_j + 1)
        if _col // _BANK != (_col + _NJ - 1) // _BANK:
            _col = ((_col + _BANK - 1) // _BANK) * _BANK
        _cols.append((_j, _col))
        _col += _NJ
    _GROUP_COLS.append(_cols)
    _GROUP_W.append(_col)

# acc/outblob layout: gap-free prefix-sum of NJ in processing order
_ACC_COL = {}       # j -> acc column
_GROUP_ACOL = []    # g -> (start col, width)
_acol = 0
for _g in range(N_GROUPS):
    _g0 = _acol
    for _j in _GROUPS[_g]:
        _ACC_COL[_j] = _acol
        _acol += 8 * (_j + 1)
    _GROUP_ACOL.append((_g0, _acol - _g0))
TOT_W = _acol
OUT_ELEMS = B * TOT_W

_compiled_nc = None


def _build_program():
    global _compiled_nc
    if _compiled_nc is not None:
        return _compiled_nc

    from contextlib import ExitStack

    nc = bacc.Bacc("TRN2", target_bir_lowering=False, debug=False)
    f32 = mybir.dt.float32
    mm_dt = {
        "f32": f32,
        "f32r": mybir.dt.float32r,
        "bf16": mybir.dt.bfloat16,
    }[KCFG["compute"]]
    out_dt = {"f32": f32, "bf16": mybir.dt.bfloat16}[KCFG["out"]]
    blob = nc.dram_tensor("blob", [BLOB_ELEMS], mm_dt, kind="ExternalInput").ap()
    outb = nc.dram_tensor("outblob", [OUT_ELEMS], out_dt, kind="ExternalOutput").ap()

    with tile.TileContext(nc) as tc, ExitStack() as ctx:
        win_pool = ctx.enter_context(tc.tile_pool(name="win", bufs=1))
        acc_pool = ctx.enter_context(tc.tile_pool(name="acc", bufs=1))
        psum_pool = ctx.enter_context(
            tc.tile_pool(name="psum", bufs=KCFG["psum_bufs"], space="PSUM")
        )

        # window 0 on sync, window 1 on scalar (its own ring, so the
        # first two windows land concurrently); the rest alternate
        # gpsimd/sync. The scalar engine afterwards only runs copies.
        win_tiles = []
        for w in range(N_WINS):
            wf = _WIN_W[w]
            t = win_pool.tile([128, wf], mm_dt, name=f"win{w}", tag=f"w{w}")
            src = blob[_WIN_OFF[w] : _WIN_OFF[w] + 128 * wf].rearrange(
                "(p f) -> p f", p=128, f=wf
            )
            if w == 0:
                q = nc.sync
            elif w == 1:
                q = nc.scalar
            else:
                q = nc.gpsimd if w % 2 == 0 else nc.sync
            q.dma_start(t[:], src)
            win_tiles.append(t)

        dstv = outb[:].rearrange("(p w) -> p w", p=B, w=TOT_W)
        sqs = [nc.sync, nc.gpsimd]
        ncopy = 0
        for g, slots in enumerate(_GROUPS):
            gw = _GROUP_W[g]
            ga, gaw = _GROUP_ACOL[g]
            psum_t = psum_pool.tile([B, gw], f32)
            for j, col in _GROUP_COLS[g]:
                NJ = 8 * (j + 1)
                wd = DCOL + NJ
                chs = _SLOT_CHUNKS[j]
                for c, (w, cb, pb, rows, _rs) in enumerate(chs):
                    t = win_tiles[w]
                    nc.tensor.matmul(
                        psum_t[:, col : col + NJ],
                        t[pb : pb + rows, cb : cb + DCOL],
                        t[pb : pb + rows, cb + DCOL : cb + wd],
                        start=(c == 0),
                        stop=(c == len(chs) - 1),
                    )
            # per-slot copies (skip psum bank-alignment gaps, land
            # gap-free in this group's own acc tile so the store's read
            # dependency is exactly these four copies), alternating
            # vector / scalar engines
            acc_g = acc_pool.tile([B, gaw], out_dt, name=f"acc{g}", tag=f"a{g}")
            for j, col in _GROUP_COLS[g]:
                NJ = 8 * (j + 1)
                ac = _ACC_COL[j] - ga
                if ncopy % 2 == 0:
                    nc.vector.tensor_copy(
                        acc_g[:, ac : ac + NJ], psum_t[:, col : col + NJ]
                    )
                else:
                    nc.scalar.copy(acc_g[:, ac : ac + NJ], psum_t[:, col : col + NJ])
                ncopy += 1
            # per-group store: streams out as soon as its copies land.
            # Dispatch stalls on sync/gpsimd are harmless (their loads
            # are already queued ahead).
            sqs[g % 2].dma_start(dstv[:, ga : ga + gaw], acc_g[:])

    nc.compile()
    _compiled_nc = nc
    return nc


def _np_dt():
    if KCFG["compute"] == "bf16":
        import ml_dtypes

        return ml_dtypes.bfloat16
    return np.float32


def _pack_core(k, x, W, bias):
    np_dt = _np_dt()
    blob = np.zeros(BLOB_ELEMS, np_dt)
    for j in range(N_SLOTS):
        i = N_CORES * j + k
        ni = i + 1
        NJ = 8 * (j + 1)
        wd = DCOL + NJ
        M = np.zeros((NJ, wd), np.float32)
        r = np.arange(ni)
        M[:ni, :DCOL] = x[:, r, i - r].T               # D^T[r, b]
        M[:ni, DCOL : DCOL + ni] = W[i, :ni, :ni].T    # V[r, q]
        Mc = M.astype(np_dt)
        for w, cb, pb, rows, rs in _SLOT_CHUNKS[j]:
            rl = Mc[rs : rs + rows]                    # [rows, wd]
            h, wf = _WIN_H[w], _WIN_W[w]
            img = blob[_WIN_OFF[w] : _WIN_OFF[w] + h * wf].reshape(h, wf)
            img[pb : pb + rows, cb : cb + wd] = rl
    return blob


def kernel(x, W, b):
    x = np.asarray(x, np.float32)
    W = np.asarray(W, np.float32)
    b = np.asarray(b, np.float32)

    nc = _build_program()
    in_maps = [{"blob": _pack_core(k, x, W, b)} for k in range(N_CORES)]
    res = run_bass_kernel_spmd(nc, in_maps, list(range(N_CORES)))

    y = x.copy()
    for k in range(N_CORES):
        ob = np.asarray(res.results[k]["outblob"], np.float32).reshape(B, TOT_W)
        for j in range(N_SLOTS):
            i = N_CORES * j + k
            ni = i + 1
            ac = _ACC_COL[j]
            q = np.arange(ni)
            y[:, q, i - q] = ob[:, ac : ac + ni] + b[i, :ni][None]
    return y
